# revision 18
# baseline (speedup 1.0000x reference)
"""DiT block kernel for Trainium2 (Bass/Tile), data-parallel over batch on 8 cores.

Per-core dataflow (one batch element per core; no collectives needed):
  - residual stream X [128 tok, 8, 768] fp32 in SBUF, updated in place
  - weights are pre-laid-out and pre-cast on host: attention Q/K/V weights in
    fp8e4 (feature-major chunk layout, contiguous per partition), FFN W1/W2 in
    bf16 -> DMA is layout-free and 2-4x smaller, no on-device casts
  - LayerNorm token-major (bn_stats/bn_aggr) -> xhat fp8 (attn) / bf16 (ffn)
    -> PE-transpose to feature-major XHT [128 d, 6, 1024 tok]
  - Q/K projections: fp8 DoubleRow matmuls (contraction 256/instr, 0.5 cyc/row)
  - scoresT per (k-tile, head): bf16 row-located matmuls -> exp on ACT
    (scale=1/8 folded; logits O(1), no max subtraction) -> fp8 exp tiles
  - attn@V: fp8 DoubleRow with the softmax denominator FOLDED INTO the
    stationary: V2[key, head, 0:64]=V, V2[key, head, 64:128]=1.0 so psum rows
    0-63 = attn numerator, rows 64-127 = denominator -- one stream instead of
    two; normalize via DVE reciprocal+mul; PE-transpose back into X
  - FFN stays bf16 for precision (fp8 would eat the whole error budget):
    h1 feature-major per ff tile, Silu on ACT -> H2 bf16; W2 bf16 resident;
    residual fused into the psum eviction
"""

import os
import sys

import numpy as np

for _p in ("/opt/trn_rl_repo", "/root/.axon_site/_ro/trn_rl_repo"):
    if os.path.isdir(_p) and _p not in sys.path:
        sys.path.insert(0, _p)

import ml_dtypes

import concourse.bass as bass
import concourse.mybir as mybir
import concourse.tile as tile
from concourse import bacc
from concourse.bass_utils import run_bass_kernel_spmd
from concourse.masks import make_identity

F32 = mybir.dt.float32
BF16 = mybir.dt.bfloat16
F8 = mybir.dt.float8e4
AF = mybir.ActivationFunctionType
OP = mybir.AluOpType
DR = mybir.MatmulPerfMode.DoubleRow

F8NP = ml_dtypes.float8_e4m3
BF16NP = ml_dtypes.bfloat16

B, T, TC, D, H, HS, FF = 8, 1024, 256, 768, 12, 64, 3072
P = 128
NT = T // P      # 8 token tiles
NTC = TC // P    # 2 context token tiles
ND = D // P      # 6 feature tiles
NF = FF // P     # 24 ffn tiles
NP = H // 2      # 6 head pairs
NKP = NT // 2    # 4 key-tile pairs (self)
EPS = 1e-5
SCALE = HS ** -0.5

WEIGHT_NAMES = [
    "ln1_w", "ln1_b", "sWq", "sbq", "sWk", "sbk", "sWv", "sbv",
    "ln2_w", "ln2_b", "cWq", "cbq", "cWk", "cbk", "cWv", "cbv",
    "ln3_w", "ln3_b", "W1", "b1", "W2", "b2",
]


def _prep_weights(inputs):
    """Host-side cast + relayout so DMA is contiguous and cast-free."""
    w = {}
    for nm in ["sWq", "sWk", "cWq", "cWk"]:
        W = np.asarray(inputs[nm], np.float32)          # [H, D, HS]
        # [g, p, dt, i*HS+e] = W[2g+i, dt*128+p, e]
        A = W.reshape(NP, 2, ND, P, HS).transpose(0, 3, 2, 1, 4)
        w[nm] = np.ascontiguousarray(
            A.reshape(NP, P, ND, P).astype(F8NP))
    for nm in ["sWv", "cWv"]:
        W = np.asarray(inputs[nm], np.float32)          # [H, D, HS]
        # [p, dt, h*HS+e] = W[h, dt*128+p, e]
        A = W.reshape(H, ND, P, HS).transpose(2, 1, 0, 3)
        w[nm] = np.ascontiguousarray(A.reshape(P, ND, D).astype(F8NP))
    W1 = np.asarray(inputs["W1"], np.float32)           # [D, FF] -> [P, ND, FF]
    w["W1"] = np.ascontiguousarray(
        W1.reshape(ND, P, FF).transpose(1, 0, 2).astype(BF16NP))
    W2 = np.asarray(inputs["W2"], np.float32)           # [FF, D] -> [P, NF, D]
    w["W2"] = np.ascontiguousarray(
        W2.reshape(NF, P, D).transpose(1, 0, 2).astype(BF16NP))
    return w


def _build(flags):
    nc = bacc.Bacc("TRN2", target_bir_lowering=False, debug=False)

    d_img = nc.dram_tensor("img_embedding", [T, D], F32, kind="ExternalInput")
    d_ctx = nc.dram_tensor("context", [TC, D], F32, kind="ExternalInput")
    dw = {}
    for i in (1, 2, 3):
        dw[f"ln{i}_w"] = nc.dram_tensor(f"ln{i}_w", [D], F32, kind="ExternalInput")
        dw[f"ln{i}_b"] = nc.dram_tensor(f"ln{i}_b", [D], F32, kind="ExternalInput")
    for nm in ["sWq", "sWk", "cWq", "cWk"]:
        dw[nm] = nc.dram_tensor(nm, [NP, P, ND, P], F8, kind="ExternalInput")
    for nm in ["sWv", "cWv"]:
        dw[nm] = nc.dram_tensor(nm, [P, ND, D], F8, kind="ExternalInput")
    for nm in ["sbq", "sbk", "sbv", "cbq", "cbk", "cbv"]:
        dw[nm] = nc.dram_tensor(nm, [H, HS], F32, kind="ExternalInput")
    dw["W1"] = nc.dram_tensor("W1", [P, ND, FF], BF16, kind="ExternalInput")
    dw["b1"] = nc.dram_tensor("b1", [FF], F32, kind="ExternalInput")
    dw["W2"] = nc.dram_tensor("W2", [P, NF, D], BF16, kind="ExternalInput")
    dw["b2"] = nc.dram_tensor("b2", [D], F32, kind="ExternalInput")
    d_out = nc.dram_tensor("out", [T, D], F32, kind="ExternalOutput")
    out_ap = d_out.ap().rearrange("(n p) d -> p n d", p=P)

    with tile.TileContext(nc) as tc, (
        tc.tile_pool(name="const", bufs=1)
    ) as const, (
        tc.tile_pool(name="resid", bufs=1)
    ) as resid, (
        tc.tile_pool(name="wpool", bufs=2)
    ) as wpool, (
        tc.tile_pool(name="big", bufs=1)
    ) as big, (
        tc.tile_pool(name="small", bufs=2)
    ) as small, (
        tc.tile_pool(name="stats", bufs=3)
    ) as stats, (
        tc.tile_pool(name="ps", bufs=1, space="PSUM")
    ) as ps:

        # ---- constants ---------------------------------------------------
        idb = const.tile([P, P], BF16)
        make_identity(nc, idb)
        eps_t = const.tile([P, 1], F32)
        nc.vector.memset(eps_t[:], EPS)

        def bcast_row(dram_ap, n):
            t = const.tile([P, n], F32)
            src = bass.AP(tensor=dram_ap.tensor, offset=dram_ap.offset,
                          ap=[[0, P]] + list(dram_ap.ap))
            nc.gpsimd.dma_start(t[:], src)
            return t

        ln_w_t, ln_b_t = {}, {}
        for i in (1, 2, 3):
            if not flags[f"ln{i}_w_triv"]:
                ln_w_t[i] = bcast_row(dw[f"ln{i}_w"].ap(), D)
            if not flags[f"ln{i}_b_triv"]:
                ln_b_t[i] = bcast_row(dw[f"ln{i}_b"].ap(), D)
        b2_t = None if flags["b2_zero"] else bcast_row(dw["b2"].ap(), D)

        def pair_bias(nm):
            t = const.tile([P, NP], F32)
            nc.sync.dma_start(
                t[:], dw[nm].ap().rearrange("(g i) e -> (i e) g", i=2))
            return t

        sbq_t = None if flags["sbq_zero"] else pair_bias("sbq")
        sbk_t = None if flags["sbk_zero"] else pair_bias("sbk")
        cbq_t = None if flags["cbq_zero"] else pair_bias("cbq")
        cbk_t = None if flags["cbk_zero"] else pair_bias("cbk")
        sbv_t = None if flags["sbv_zero"] else bcast_row(
            dw["sbv"].ap().rearrange("h e -> (h e)"), D)
        cbv_t = None if flags["cbv_zero"] else bcast_row(
            dw["cbv"].ap().rearrange("h e -> (h e)"), D)
        b1_t = None
        if not flags["b1_zero"]:
            b1_t = const.tile([P, NF], F32)
            nc.sync.dma_start(b1_t[:], dw["b1"].ap().rearrange("(f p) -> p f", p=P))

        # ---- input + resident-weight DMAs (issue order = urgency) -------
        ctx_st = [small.tile([P, D], F32, tag="fst", name=f"ctx{t}")
                  for t in range(NTC)]
        for t in range(NTC):
            nc.sync.dma_start(ctx_st[t][:], d_ctx.ap().rearrange(
                "(n p) d -> p n d", p=P)[:, t])

        X = resid.tile([P, NT, D], F32)
        img_t = d_img.ap().rearrange("(n p) d -> p n d", p=P)
        for t in range(NT):
            nc.sync.dma_start(X[:, t], img_t[:, t])

        wq_s = const.tile([P, NP, ND, P], F8)
        wk_s = const.tile([P, NP, ND, P], F8)
        wq_c = const.tile([P, NP, ND, P], F8)
        wk_c = const.tile([P, NP, ND, P], F8)
        wv_s = const.tile([P, ND, D], F8)
        wv_c = const.tile([P, ND, D], F8)
        for tl, nm in ((wq_s, "sWq"), (wk_s, "sWk"), (wq_c, "cWq"),
                       (wk_c, "cWk")):
            nc.sync.dma_start(tl[:], dw[nm].ap().rearrange(
                "g p dt m -> p g dt m"))
        nc.sync.dma_start(wv_s[:], dw["sWv"].ap())
        nc.sync.dma_start(wv_c[:], dw["cWv"].ap())

        # ---- context, transposed fp8 -------------------------------------
        ctxT = resid.tile([P, ND, TC], F8)
        for t in range(NTC):
            c8 = small.tile([P, D], BF16, tag="xh", name=f"c8_{t}")
            nc.vector.tensor_copy(c8[:], ctx_st[t][:])
            pt = ps.tile([P, D], BF16, tag=("sA" if t % 2 == 0 else "sB"))
            for j in range(ND):
                nc.tensor.transpose(pt[:, j * P:(j + 1) * P],
                                    c8[:, j * P:(j + 1) * P], idb[:])
            nc.vector.tensor_copy(ctxT[:, :, t * P:(t + 1) * P], pt[:].rearrange(
                "p (j q) -> p j q", q=P))

        # ---- helpers -----------------------------------------------------
        def layernorm_to_T(i, XHT, dt8):
            for t in range(NT):
                st = stats.tile([P, 2, 6], F32, tag="bst")
                nc.vector.bn_stats(st[:, 0, :], X[:, t, 0:512])
                nc.vector.bn_stats(st[:, 1, :], X[:, t, 512:768])
                mv = stats.tile([P, 2], F32, tag="mv")
                nc.vector.bn_aggr(mv[:], st[:])
                sd = stats.tile([P, 1], F32, tag="sd")
                nc.scalar.activation(sd[:], mv[:, 1:2], AF.Sqrt, bias=eps_t[:])
                rstd = stats.tile([P, 1], F32, tag="rstd")
                nc.vector.reciprocal(rstd[:], sd[:])
                nmr = stats.tile([P, 1], F32, tag="nmr")
                nc.vector.tensor_scalar(nmr[:], mv[:, 0:1], rstd[:], -1.0,
                                        OP.mult, OP.mult)
                xh = small.tile([P, D], BF16, tag="xh", name=f"xh_{i}_{t}")
                if i in ln_w_t or i in ln_b_t:
                    xf = small.tile([P, D], F32, tag="fst", name=f"xf_{i}_{t}")
                    nc.vector.tensor_scalar(xf[:], X[:, t, :], mv[:, 0:1],
                                            rstd[:], OP.subtract, OP.mult)
                    if i in ln_w_t and i in ln_b_t:
                        nc.vector.tensor_mul(xf[:], xf[:], ln_w_t[i][:])
                        nc.vector.tensor_tensor(xh[:], xf[:], ln_b_t[i][:], OP.add)
                    elif i in ln_w_t:
                        nc.vector.tensor_tensor(xh[:], xf[:], ln_w_t[i][:], OP.mult)
                    else:
                        nc.vector.tensor_tensor(xh[:], xf[:], ln_b_t[i][:], OP.add)
                else:
                    nc.scalar.activation(xh[:], X[:, t, :], AF.Identity,
                                         bias=nmr[:], scale=rstd[:])
                if i == 3:
                    nc.sync.dma_start_transpose(XHT[:, t], xh[:])
                else:
                    pt = ps.tile([P, D], BF16,
                                 tag=("sA" if t % 2 == 0 else "sB"))
                    for j in range(ND):
                        nc.tensor.transpose(pt[:, j * P:(j + 1) * P],
                                            xh[:, j * P:(j + 1) * P], idb[:])
                    nc.scalar.copy(
                        XHT[:, :, t * P:(t + 1) * P],
                        pt[:].rearrange("p (j q) -> p j q", q=P))

        def project_v(wv, XT, n_tok, V2, bias_t):
            """V2 [P, nkp, 2, H, P] fp8: cols 0:HS = x @ Wv (token-major),
            cols HS:P stay 1.0 (softmax denominator trick)."""
            nc.gpsimd.memset(V2[:, :, :, :, HS:P], 1.0)
            for t in range(n_tok // P):
                pv = ps.tile([P, D], F32, tag="pq", name=f"pv{t}")
                for o, w in ((0, 512), (512, 256)):
                    for mp in range(ND // 2):
                        nc.tensor.matmul(
                            pv[:, o:o + w],
                            XT[:, 2 * mp:2 * mp + 2, t * P:(t + 1) * P],
                            wv[:, 2 * mp:2 * mp + 2, o:o + w],
                            start=(mp == 0), stop=(mp == ND // 2 - 1),
                            perf_mode=DR)
                dst = V2[:, t // 2, t % 2, :, 0:HS]
                src = pv[:].rearrange("p (h e) -> p h e", e=HS)
                if bias_t is not None:
                    nc.vector.tensor_tensor(
                        dst, src, bias_t[:].rearrange("p (h e) -> p h e", e=HS),
                        OP.add)
                else:
                    nc.vector.tensor_copy(dst, src)

        def attention(wq, wk, XTq, XTkv, n_kv, V2, qb, kb, tag):
            """Full attention pass; adds output into X in place.

            Software pipeline, per loop iteration g:
              scores+exp(g) | proj(g+1) | flush(g-2) | attn@V(g-1)
            attn@V is deferred one iteration so the PE never waits on the ACT
            exp backlog (exp is the phase bottleneck); exp tiles are
            double-buffered by g parity.
            """
            nk = n_kv // P
            nkp = nk // 2
            exs = [big.tile([P, 2, nkp, 2, T], F8, tag="expA",
                            name=f"exA_{tag}"),
                   big.tile([P, 2, nkp, 2, T], F8, tag="expB",
                            name=f"exB_{tag}")]
            aogs = {}

            def do_proj(g):
                pq = ps.tile([P, T], F32, tag="pq", name=f"pq_{tag}_{g}")
                for c in range(2):
                    for mp in range(ND // 2):
                        nc.tensor.matmul(
                            pq[:, c * 512:(c + 1) * 512],
                            wq[:, g, 2 * mp:2 * mp + 2, :],
                            XTq[:, 2 * mp:2 * mp + 2, c * 512:(c + 1) * 512],
                            start=(mp == 0), stop=(mp == ND // 2 - 1),
                            perf_mode=DR)
                qg = small.tile([P, T], BF16, tag="qg", name=f"qg_{tag}_{g}")
                if qb is not None:
                    if tag == "c":
                        nc.scalar.activation(qg[:], pq[:], AF.Identity,
                                             bias=qb[:, g:g + 1])
                    else:
                        nc.vector.tensor_scalar(qg[:], pq[:],
                                                qb[:, g:g + 1], None, OP.add)
                elif tag == "c":
                    nc.scalar.copy(qg[:], pq[:])
                else:
                    nc.vector.tensor_copy(qg[:], pq[:])
                pk = ps.tile([P, n_kv], F32, tag="pq", name=f"pk_{tag}_{g}")
                for c in range(max(1, n_kv // 512)):
                    w = min(512, n_kv)
                    for mp in range(ND // 2):
                        nc.tensor.matmul(
                            pk[:, c * w:(c + 1) * w],
                            wk[:, g, 2 * mp:2 * mp + 2, :],
                            XTkv[:, 2 * mp:2 * mp + 2, c * w:(c + 1) * w],
                            start=(mp == 0), stop=(mp == ND // 2 - 1),
                            perf_mode=DR)
                kg = small.tile([P, n_kv], BF16, tag="kg", name=f"kg_{tag}_{g}")
                if kb is not None:
                    nc.vector.tensor_scalar(kg[:], pk[:],
                                            kb[:, g:g + 1], None, OP.add)
                else:
                    nc.vector.tensor_copy(kg[:], pk[:])
                return qg, kg

            def do_scores(g, qg, kg):
                ex = exs[g % 2]
                for k in range(nk):
                    for i in range(2):
                        sc = ps.tile([P, T], F32,
                                     tag=("sA" if (2 * k + i) % 2 == 0 else "sB"),
                                     name=f"sc_{tag}_{g}_{k}_{i}")
                        for c in range(2):
                            nc.tensor.matmul(
                                sc[:, c * 512:(c + 1) * 512],
                                kg[i * HS:(i + 1) * HS, k * P:(k + 1) * P],
                                qg[i * HS:(i + 1) * HS, c * 512:(c + 1) * 512],
                                start=True, stop=True)
                        nc.scalar.activation(ex[:, i, k // 2, k % 2, :], sc[:],
                                             AF.Exp, scale=SCALE)

            def do_attnv(g):
                ex = exs[g % 2]
                # aog2 [hs 64, head i, c-half, 512 q] == [64, 2048] 2D
                aog = small.tile([HS, 2, 2, 512], BF16, tag="aog",
                                 name=f"aog_{tag}_{g}")
                rec = small.tile([HS, 2, T], F32, tag="rec", bufs=1,
                                 name=f"rec_{tag}_{g}")
                for c in range(2):
                    po = ps.tile([P, T], F32, tag=("avs" if c == 0 else "pq"),
                                 name=f"po_{tag}_{g}_{c}")
                    for kp in range(nkp):
                        for i in range(2):
                            nc.tensor.matmul(
                                po[:, i * 512:(i + 1) * 512],
                                V2[:, kp, :, 2 * g + i, :],
                                ex[:, i, kp, :, c * 512:(c + 1) * 512],
                                start=(kp == 0), stop=(kp == nkp - 1),
                                perf_mode=DR, skip_group_check=True)
                    nc.vector.reciprocal(rec[:, c, :], po[HS:P, :])
                    nc.vector.tensor_tensor(
                        aog[:, :, c, :],
                        po[0:HS, :].rearrange("p (i q) -> p i q", q=512),
                        rec[:, c, :].rearrange("p (i q) -> p i q", q=512),
                        OP.mult)
                aogs[g] = aog

            def do_flush(g):
                aog = aogs.pop(g)
                ftr = small.tile([P, 2 * NT, HS], BF16, tag="ftr",
                                 name=f"ftr_{tag}_{g}")
                nc.sync.dma_start_transpose(
                    ftr[:], aog[:].rearrange("p i c q -> p (i c q)"))
                eng = nc.gpsimd if g < NP - 2 else nc.vector
                for i in range(2):
                    xv = X[:, :, g * P + i * HS:g * P + (i + 1) * HS]
                    eng.tensor_tensor(xv, ftr[:, i * NT:(i + 1) * NT, :], xv,
                                      OP.add)

            carry = do_proj(0)
            for g in range(NP):
                qg, kg = carry
                do_scores(g, qg, kg)
                if g + 1 < NP:
                    carry = do_proj(g + 1)
                if g - 2 >= 0:
                    do_flush(g - 2)
                if g - 1 >= 0:
                    do_attnv(g - 1)
            do_attnv(NP - 1)
            do_flush(NP - 2)
            do_flush(NP - 1)

        # =================== self attention ==============================
        XHT = big.tile([P, ND, T], F8, tag="xht", name="xht1")
        layernorm_to_T(1, XHT, F8)
        V2s = big.tile([P, NKP, 2, H, P], F8, tag="vw2", name="v2s")
        project_v(wv_s, XHT, T, V2s, sbv_t)
        attention(wq_s, wk_s, XHT, XHT, T, V2s, sbq_t, sbk_t, "s")

        # =================== cross attention =============================
        XHT2 = big.tile([P, ND, T], F8, tag="xht", name="xht2")
        layernorm_to_T(2, XHT2, F8)
        V2c = big.tile([P, NTC // 2, 2, H, P], F8, tag="vc", name="v2c")
        project_v(wv_c, ctxT, TC, V2c, cbv_t)

        # W2 resident load (fills DMA idle during cross attention; vw2 tag
        # is free once self-attention has consumed V2s)
        W2b = big.tile([P, NF, D], BF16, tag="vw2", name="w2b")
        for q4 in range(4):
            nc.sync.dma_start(W2b[:, 6 * q4:6 * q4 + 6, :],
                              dw["W2"].ap()[:, 6 * q4:6 * q4 + 6, :])

        attention(wq_c, wk_c, XHT2, ctxT, TC, V2c, cbq_t, cbk_t, "c")

        # =================== FFN =========================================
        XHT3 = big.tile([P, NT, ND, P], BF16, tag="xht", name="xht3")
        layernorm_to_T(3, XHT3, BF16)

        NFH = NF // 2
        H2a = big.tile([P, NFH, T], BF16, tag="expA", name="h2a")
        H2b = big.tile([P, NFH, T], BF16, tag="expB", name="h2b")
        for fp_ in range(NF // 2):
            w1b = wpool.tile([P, ND, 2 * P], BF16, tag="w1b", name=f"w1b_{fp_}")
            nc.sync.dma_start(
                w1b[:], dw["W1"].ap()[:, :, fp_ * 2 * P:(fp_ + 1) * 2 * P])
            for i in range(2):
                f = fp_ * 2 + i
                ph = ps.tile([P, T], F32,
                             tag=["avs", "pq", "sA", "sB"][f % 4],
                             name=f"ph_{f}")
                for c in range(2):
                    for dt in range(ND):
                        nc.tensor.matmul(
                            ph[:, c * 512:(c + 1) * 512],
                            w1b[:, dt, i * P:(i + 1) * P],
                            XHT3[:, 4 * c:4 * c + 4, dt, :],
                            start=(dt == 0), stop=(dt == ND - 1))
                h2dst = (H2a[:, f, :] if f < NFH else H2b[:, f - NFH, :])
                nc.scalar.activation(
                    h2dst, ph[:], AF.Silu,
                    bias=(b1_t[:, f:f + 1] if b1_t is not None else 0.0))

        for t in range(NT):
            pf = ps.tile([P, D], F32,
                         tag=["sA", "sB", "avs", "pq"][t % 4],
                         name=f"pf_{t}")
            for o, w in ((0, 512), (512, 256)):
                for f in range(NF):
                    h2src = (H2a[:, f, t * P:(t + 1) * P] if f < NFH
                             else H2b[:, f - NFH, t * P:(t + 1) * P])
                    nc.tensor.matmul(
                        pf[:, o:o + w],
                        h2src,
                        W2b[:, f, o:o + w],
                        start=(f == 0), stop=(f == NF - 1))
            ot = small.tile([P, D], F32, tag="ot", name=f"ot_{t}")
            nc.vector.tensor_tensor(ot[:], pf[:], X[:, t, :], OP.add)
            if b2_t is not None:
                nc.vector.tensor_add(ot[:], ot[:], b2_t[:])
            nc.sync.dma_start(out_ap[:, t], ot[:])

    nc.compile()
    return nc


_CACHE = {}


def _flags_of(inputs):
    f = {}
    for i in (1, 2, 3):
        f[f"ln{i}_w_triv"] = bool(np.all(np.asarray(inputs[f"ln{i}_w"]) == 1.0))
        f[f"ln{i}_b_triv"] = bool(np.all(np.asarray(inputs[f"ln{i}_b"]) == 0.0))
    for nm in ["sbq", "sbk", "sbv", "cbq", "cbk", "cbv", "b1", "b2"]:
        f[f"{nm}_zero"] = bool(np.all(np.asarray(inputs[nm]) == 0.0))
    return f


def kernel(**inputs):
    flags = _flags_of(inputs)
    key = tuple(sorted(flags.items()))
    if key not in _CACHE:
        _CACHE[key] = _build(flags)
    nc = _CACHE[key]

    wprep = _prep_weights(inputs)
    base = {}
    for nm in WEIGHT_NAMES:
        base[nm] = wprep.get(nm, np.ascontiguousarray(
            np.asarray(inputs[nm], np.float32)))

    in_maps = []
    for b in range(B):
        m = dict(base)
        m["img_embedding"] = np.ascontiguousarray(
            np.asarray(inputs["img_embedding"][b], np.float32))
        m["context"] = np.ascontiguousarray(
            np.asarray(inputs["context"][b], np.float32))
        in_maps.append(m)

    res = run_bass_kernel_spmd(nc, in_maps, core_ids=list(range(B)))
    return np.stack([res.results[b]["out"] for b in range(B)], axis=0)
